# revision 8
# baseline (speedup 1.0000x reference)
"""CNN char encoder (conv widths 1/2/3 -> tanh -> max over time -> highway)
as a Bass/Tile kernel for 8 Trainium2 NeuronCores.

Sharding: data-parallel over the 4096 = 32*128 flattened words; 512 words per
core; all weights replicated. Everything on-chip is feature-major
([feature_partition, word] tiles); the host does the final transpose back to
(B, S, OUT_DIM).

Input is shipped in a "pair" layout
    xp[r, n*10 + u] = x[word n, char position 2u+d, channel c]
with r = c for d=0 and r = 64+c for d=1 (rows 50..63 zero padding, because
matmul operand base partitions must be 0/32/64 and must match between lhsT
and rhs). A width-3 conv output position then needs 2 matmuls (K=114 + K=50)
instead of 3 (one per tap); width-1/2 convs read row-blocks of the same tile.

Conv positions are evaluated one t per PSUM tile [128, 512 words]. Max over
time is tanh-first (tanh is monotone; conv bias folded into the ACT op) and
is split between two routes to balance ACT vs DVE load:
  - ACT route: tanh PSUM -> bf16 SBUF scratch, DVE bf16 max-acc (2x mode)
  - DVE route: fp32 max straight from PSUM, single tanh at the end
Conv matmuls are float32r (full PE rate at moving dim 512); the highway
matmuls run in bf16 (feat tiles and highway weights are bf16), with the
highway epilogue t*(h-f)+f in fp32.
"""

import numpy as np
import ml_dtypes

import concourse.bass as bass
import concourse.tile as tile
from concourse import bacc, mybir
from concourse.bass_utils import run_bass_kernel_spmd

F32 = mybir.dt.float32
F32R = mybir.dt.float32r
BF16 = mybir.dt.bfloat16
ACTF = mybir.ActivationFunctionType

N_CORES = 8
B, S, L, C = 32, 128, 20, 50
NW = B * S               # 4096 words total
WPC = NW // N_CORES      # 512 words per core
U = L // 2               # 10 pairs per word
D1 = 64                  # partition base of the d=1 block
R = D1 + C               # 114 rows in the pair layout
OUT_DIM = 768


def _routes(n, n_act):
    """Spread n_act ACT-routed positions evenly through a bank."""
    if n_act <= 0:
        return [False] * n
    step = n / n_act
    picks = {int(i * step) for i in range(n_act)}
    return [i in picks for i in range(n)]


# tuning knobs: how many of each bank's positions go down the ACT+bf16-DVE
# route (the rest take the fp32-from-PSUM DVE route).
ACT_TILES = {"w1": 15, "w2": 14, "w3_0": 14, "w3_1": 14, "w3_2": 14, "w3_3": 14}


CONV_ONLY = False
BANKS = ("w1", "w2", "w3_0", "w3_1", "w3_2", "w3_3")


def build_nc():
    nc = bacc.Bacc(
        "TRN2", target_bir_lowering=False, debug=False, num_devices=N_CORES
    )

    xp = nc.dram_tensor("xp", [R, WPC * U], F32, kind="ExternalInput")
    w1p = nc.dram_tensor("w1p", [R, 128], F32, kind="ExternalInput")
    w2p = nc.dram_tensor("w2p", [R, 128], F32, kind="ExternalInput")
    w2o = nc.dram_tensor("w2o", [R, 128], F32, kind="ExternalInput")
    w2z = nc.dram_tensor("w2z", [R, 128], F32, kind="ExternalInput")
    w3a = nc.dram_tensor("w3a", [R, 512], F32, kind="ExternalInput")
    w3t2 = nc.dram_tensor("w3t2", [C, 512], F32, kind="ExternalInput")
    w3o = nc.dram_tensor("w3o", [R, 512], F32, kind="ExternalInput")
    w3d = nc.dram_tensor("w3d", [R, 512], F32, kind="ExternalInput")
    biasp = nc.dram_tensor("biasp", [128, 18], F32, kind="ExternalInput")
    whp = nc.dram_tensor("whp", [128, 36 * 128], BF16, kind="ExternalInput")
    wtp = nc.dram_tensor("wtp", [128, 36 * 128], BF16, kind="ExternalInput")
    out_dt = BF16 if CONV_ONLY else F32
    out = nc.dram_tensor("out", [OUT_DIM, WPC], out_dt, kind="ExternalOutput")

    with tile.TileContext(nc) as tc:
        with (
            tc.tile_pool(name="singles", bufs=1) as singles,
            tc.tile_pool(name="psum", bufs=6, space="PSUM") as psum,
            tc.tile_pool(name="gscr", bufs=4) as gscr,
            tc.tile_pool(name="hwtiles", bufs=2) as hwt,
        ):
            sb_x = singles.tile([R, WPC * U], F32R)
            nc.sync.dma_start(out=sb_x, in_=xp.ap().bitcast(F32R))
            sb_w1 = singles.tile([R, 128], F32R)
            nc.sync.dma_start(out=sb_w1, in_=w1p.ap().bitcast(F32R))
            sb_w2 = singles.tile([R, 128], F32R)
            nc.sync.dma_start(out=sb_w2, in_=w2p.ap().bitcast(F32R))
            sb_w2o = singles.tile([R, 128], F32R)
            nc.sync.dma_start(out=sb_w2o, in_=w2o.ap().bitcast(F32R))
            sb_w2z = singles.tile([R, 128], F32R)
            nc.sync.dma_start(out=sb_w2z, in_=w2z.ap().bitcast(F32R))
            sb_w3a = singles.tile([R, 512], F32R)
            nc.sync.dma_start(out=sb_w3a, in_=w3a.ap().bitcast(F32R))
            sb_w3t2 = singles.tile([C, 512], F32R)
            nc.sync.dma_start(out=sb_w3t2, in_=w3t2.ap().bitcast(F32R))
            sb_w3o = singles.tile([R, 512], F32R)
            nc.sync.dma_start(out=sb_w3o, in_=w3o.ap().bitcast(F32R))
            sb_w3d = singles.tile([R, 512], F32R)
            nc.sync.dma_start(out=sb_w3d, in_=w3d.ap().bitcast(F32R))
            sb_bias = singles.tile([128, 18], F32)
            nc.sync.dma_start(out=sb_bias, in_=biasp.ap())
            sb_wh = singles.tile([128, 36 * 128], BF16)
            nc.sync.dma_start(out=sb_wh, in_=whp.ap())
            sb_wt = singles.tile([128, 36 * 128], BF16)
            nc.sync.dma_start(out=sb_wt, in_=wtp.ap())

            # [R, words, pairs] view for strided column access
            xv = sb_x.rearrange("p (n u) -> p n u", u=U)

            def xcols(row0, nrows, u):
                return xv[row0 : row0 + nrows, :, u]

            feat = []
            for j in range(6):
                fj = singles.tile([128, WPC], BF16, name=f"feat{j}")
                feat.append(fj)

            def conv_bank(name, feat_tile, bias_col, positions, n_act):
                """positions: list (per t) of lists of (lhsT_ap, rhs_ap)."""
                routes = _routes(len(positions), n_act)
                acc = None          # DVE route accumulator (pre-tanh max)
                acc_started = False
                act_started = False
                if any(not r for r in routes):
                    acc = singles.tile([128, WPC], F32, name=f"acc_{name}")
                for idx, mms in enumerate(positions):
                    y = psum.tile([128, WPC], F32, name="ypsum", bufs=6)
                    nmm = len(mms)
                    for i, (lhsT, rhs) in enumerate(mms):
                        nc.tensor.matmul(
                            y, lhsT, rhs, start=(i == 0), stop=(i == nmm - 1)
                        )
                    if routes[idx]:
                        if not act_started:
                            nc.scalar.activation(
                                feat_tile, y, ACTF.Tanh, bias=bias_col
                            )
                            act_started = True
                        else:
                            scr = gscr.tile(
                                [128, WPC], BF16, name="gscr_t", bufs=4
                            )
                            nc.scalar.activation(scr, y, ACTF.Tanh, bias=bias_col)
                            nc.vector.tensor_max(feat_tile, feat_tile, scr)
                    else:
                        if not acc_started:
                            nc.vector.tensor_copy(acc, y)
                            acc_started = True
                        else:
                            nc.vector.tensor_max(acc, acc, y)
                # merge the two routes
                if acc_started and act_started:
                    mrg = gscr.tile([128, WPC], BF16, name="gscr_t", bufs=4)
                    nc.scalar.activation(mrg, acc, ACTF.Tanh, bias=bias_col)
                    nc.vector.tensor_max(feat_tile, feat_tile, mrg)
                elif acc_started:
                    nc.scalar.activation(feat_tile, acc, ACTF.Tanh, bias=bias_col)

            # width-1 bank -> feat[0]; position t = 2u+d reads row-block d
            pos = []
            for t in range(20):
                d, u = t % 2, t // 2
                r0 = 0 if d == 0 else D1
                pos.append([(sb_w1[r0 : r0 + C, :], xcols(r0, C, u))])
            if "w1" in BANKS:
                conv_bank("w1", feat[0], sb_bias[:, 0:1], pos, ACT_TILES["w1"])

            # width-2 bank -> feat[1]
            pos = []
            for t in range(19):
                if t % 2 == 0:
                    u = t // 2
                    pos.append([(sb_w2[0:R, :], xcols(0, R, u))])
                else:
                    u = (t - 1) // 2
                    pos.append(
                        [
                            (sb_w2z[0:R, :], xcols(0, R, u)),
                            (sb_w2o[0:C, :], xcols(0, C, u + 1)),
                        ]
                    )
            if "w2" in BANKS:
                conv_bank("w2", feat[1], sb_bias[:, 1:2], pos, ACT_TILES["w2"])

            # width-3 banks -> feat[2..5]
            for j in range(4):
                cs = slice(j * 128, (j + 1) * 128)
                aj = sb_w3a[:, cs]
                t2j = sb_w3t2[:, cs]
                oj = sb_w3o[:, cs]
                dj = sb_w3d[:, cs]
                pos = []
                for t in range(18):
                    if t % 2 == 0:
                        u = t // 2
                        pos.append(
                            [
                                (aj[0:R, :], xcols(0, R, u)),
                                (t2j[0:C, :], xcols(0, C, u + 1)),
                            ]
                        )
                    else:
                        u = (t - 1) // 2
                        pos.append(
                            [
                                (oj[0:R, :], xcols(0, R, u)),
                                (dj[0:R, :], xcols(0, R, u + 1)),
                            ]
                        )
                if f"w3_{j}" in BANKS:
                    conv_bank(
                        f"w3_{j}",
                        feat[2 + j],
                        sb_bias[:, 2 + j : 3 + j],
                        pos,
                        ACT_TILES[f"w3_{j}"],
                    )

            if CONV_ONLY:
                bank_of = {0: "w1", 1: "w2", 2: "w3_0", 3: "w3_1", 4: "w3_2", 5: "w3_3"}
                for j in range(6):
                    if bank_of[j] in BANKS:
                        nc.sync.dma_start(
                            out=out.ap()[j * 128 : (j + 1) * 128, :], in_=feat[j]
                        )
            # highway: h = relu(Wh f + bh), t = sig(Wt f + bt),
            # out = t*(h-f) + f, all feature-major [128 out-feats, 512 words]
            for ot in range(() if CONV_ONLY else range(6)) if False else (range(0) if CONV_ONLY else range(6)):
                hp = psum.tile([128, WPC], F32, name="ypsum", bufs=6)
                for kt in range(6):
                    blk = (ot * 6 + kt) * 128
                    nc.tensor.matmul(
                        hp,
                        sb_wh[:, blk : blk + 128],
                        feat[kt],
                        start=(kt == 0),
                        stop=(kt == 5),
                    )
                tp = psum.tile([128, WPC], F32, name="ypsum", bufs=6)
                for kt in range(6):
                    blk = (ot * 6 + kt) * 128
                    nc.tensor.matmul(
                        tp,
                        sb_wt[:, blk : blk + 128],
                        feat[kt],
                        start=(kt == 0),
                        stop=(kt == 5),
                    )
                h_sb = hwt.tile([128, WPC], F32, name="h_sb", bufs=2)
                nc.scalar.activation(
                    h_sb, hp, ACTF.Relu, bias=sb_bias[:, 6 + ot : 7 + ot]
                )
                t_sb = hwt.tile([128, WPC], F32, name="t_sb", bufs=2)
                nc.scalar.activation(
                    t_sb, tp, ACTF.Sigmoid, bias=sb_bias[:, 12 + ot : 13 + ot]
                )
                out_sb = hwt.tile([128, WPC], F32, name="out_sb", bufs=2)
                nc.vector.tensor_sub(h_sb, h_sb, feat[ot])
                nc.vector.tensor_mul(h_sb, t_sb, h_sb)
                nc.vector.tensor_add(out_sb, h_sb, feat[ot])
                nc.sync.dma_start(
                    out=out.ap()[ot * 128 : (ot + 1) * 128, :], in_=out_sb
                )

    nc.compile()
    return nc


def pack_inputs(ts10_input, conv_w0, conv_b0, conv_w1, conv_b1, conv_w2,
                conv_b2, wh_w, wh_b, wt_w, wt_b):
    f = np.float32

    def padded(top, bottom, ncols):
        arr = np.zeros((R, ncols), f)
        if top is not None:
            arr[0:C] = top
        if bottom is not None:
            arr[D1:R] = bottom
        return arr

    X = np.ascontiguousarray(ts10_input, dtype=f).reshape(NW, L, C)
    w1t = conv_w0[:, :, 0].T
    w1p = padded(w1t, w1t, 128)
    w2p = padded(conv_w1[:, :, 0].T, conv_w1[:, :, 1].T, 128)
    w2o = padded(conv_w1[:, :, 1].T, conv_w1[:, :, 0].T, 128)
    w2z = padded(None, conv_w1[:, :, 0].T, 128)
    w3a = padded(conv_w2[:, :, 0].T, conv_w2[:, :, 1].T, 512)
    w3t2 = np.ascontiguousarray(conv_w2[:, :, 2].T, dtype=f)
    w3o = padded(None, conv_w2[:, :, 0].T, 512)
    w3d = padded(conv_w2[:, :, 1].T, conv_w2[:, :, 2].T, 512)
    biasp = np.zeros((128, 18), f)
    biasp[:, 0] = conv_b0
    biasp[:, 1] = conv_b1
    for j in range(4):
        biasp[:, 2 + j] = conv_b2[j * 128 : (j + 1) * 128]
    for ot in range(6):
        biasp[:, 6 + ot] = wh_b[ot * 128 : (ot + 1) * 128]
        biasp[:, 12 + ot] = wt_b[ot * 128 : (ot + 1) * 128]
    whp = np.ascontiguousarray(
        wh_w.reshape(6, 128, 6, 128).transpose(3, 0, 2, 1).reshape(128, 36 * 128)
    ).astype(ml_dtypes.bfloat16)
    wtp = np.ascontiguousarray(
        wt_w.reshape(6, 128, 6, 128).transpose(3, 0, 2, 1).reshape(128, 36 * 128)
    ).astype(ml_dtypes.bfloat16)
    shared = dict(w1p=w1p, w2p=w2p, w2o=w2o, w2z=w2z, w3a=w3a, w3t2=w3t2, w3o=w3o,
                  w3d=w3d, biasp=biasp, whp=whp, wtp=wtp)
    in_maps = []
    for c in range(N_CORES):
        Xc = X[c * WPC : (c + 1) * WPC]            # [512, 20, 50]
        pair = Xc.reshape(WPC, U, 2, C).transpose(2, 3, 0, 1)  # [2, C, 512, U]
        xpc = np.zeros((R, WPC * U), f)
        xpc[0:C] = pair[0].reshape(C, WPC * U)
        xpc[D1:R] = pair[1].reshape(C, WPC * U)
        in_maps.append(dict(xp=xpc, **shared))
    return in_maps


_NC_CACHE = None


def get_nc():
    global _NC_CACHE
    if _NC_CACHE is None:
        _NC_CACHE = build_nc()
    return _NC_CACHE


def kernel(**inputs):
    in_maps = pack_inputs(**{k: np.asarray(v) for k, v in inputs.items()})
    nc = get_nc()
    res = run_bass_kernel_spmd(nc, in_maps, core_ids=list(range(N_CORES)))
    full = np.empty((NW, OUT_DIM), np.float32)
    for c in range(N_CORES):
        full[c * WPC : (c + 1) * WPC] = res.results[c]["out"].T
    return full.reshape(B, S, OUT_DIM)
